# revision 9
# baseline (speedup 1.0000x reference)
"""Trainium2 Bass kernel for nn_AlignBinary (token-equality similarity).

Reference semantics: with emb_weight fixed to the identity matrix, the
one-hot bmm + mask reduces exactly to

    out[b, q, c] = 1.0 if (qry[b,q] == cnd[b,c] and qry[b,q] > 0) else 0.0

Strategy (pure data parallel, batch B=128 split over 8 cores, 16 each):
  - remap qry zeros to -1 on device (qry' = qry - (qry==0)); then a single
    is_equal(qry'[q], cnd[c]) realizes sim * mask (a -1 never matches a
    cnd value in [0, 1023], and equal nonzero pairs imply both masks set).
  - per batch: broadcast the cnd row across 128 partitions with a K=16
    row-selector matmul on the PE, then one DVE tensor_scalar is_equal with
    the per-partition qry' column produces the [128, 128] f32 tile.
  - qry, cnd and the f32 constants (bit-viewed as int32) ride ONE packed
    input tensor / one DMA: compute instructions and the tail drain only
    support a small number of sync waits, so we keep the distinct
    semaphore lane count low.
"""

import numpy as np

B = 128
L = 128
N_CORES = 8
B_LOC = B // N_CORES  # 16 batches per core

# packed input layout (int32, per core): [16, 128 + 128 + 2064]
#   [:, 0:128]        qry rows
#   [:, 128:256]      cnd rows
#   [:, 256:2304]     bigsel (f32 bits): bigsel[k, b*L + q] = (k == b)
#   [:, 2304:2320]    eye(16) (f32 bits)
_QOFF = 0
_COFF = L
_SELOFF = 2 * L
_IDOFF = _SELOFF + B_LOC * L
_PACKW = _IDOFF + B_LOC

_CACHE: dict = {}


def _consts_i32():
    ident = np.eye(B_LOC, dtype=np.float32)
    bigsel = np.repeat(ident[:, :, None], L, axis=2).reshape(B_LOC, -1)
    return np.concatenate([bigsel, ident], axis=1).view(np.int32)


def _build_nc():
    import concourse.bass as bass
    import concourse.mybir as mybir
    from concourse import tile

    dt = mybir.dt
    nc = bass.Bass(trn_type="TRN2", name="align_binary")

    packed_d = nc.dram_tensor("packed", [B_LOC, _PACKW], dt.int32, kind="ExternalInput")
    out_d = nc.dram_tensor("out", [B_LOC, L, L], dt.float32, kind="ExternalOutput")

    with tile.TileContext(nc) as tc:
        with (
            tc.tile_pool(name="sbuf", bufs=1) as pool,
            tc.tile_pool(name="psum", bufs=4, space=bass.MemorySpace.PSUM) as psum,
            tc.tile_pool(name="psum1", bufs=1, space=bass.MemorySpace.PSUM) as psum1,
        ):
            # --- one DMA for everything ---
            big = pool.tile([B_LOC, _PACKW], dt.int32)
            nc.sync.dma_start(out=big[:], in_=packed_d[:])
            qi = big[:, _QOFF:_QOFF + L]
            ci = big[:, _COFF:_COFF + L]
            bigsel = big[:, _SELOFF:_IDOFF].bitcast(dt.float32).rearrange(
                "k (b q) -> k b q", b=B_LOC
            )
            ident = big[:, _IDOFF:_PACKW].bitcast(dt.float32)

            # absorber: PE instructions carry only one sync wait; burn the
            # input-DMA wait on a throwaway matmul so later matmuls only
            # ever wait on the DVE cast semaphore.
            trash = psum1.tile([B_LOC, 1], dt.float32)
            nc.tensor.matmul(trash[:], lhsT=ident[:], rhs=ident[:, :1],
                             start=True, stop=True)

            # --- cast ids to f32 (values <= 1023, exact) ---
            qf = pool.tile([B_LOC, L], dt.float32)
            cf = pool.tile([B_LOC, L], dt.float32)
            nc.vector.tensor_copy(out=qf[:], in_=qi)
            nc.vector.tensor_copy(out=cf[:], in_=ci)

            # --- transpose qf via PE: qT[l, b] = qf[b, l] ---
            qT = psum1.tile([L, B_LOC], dt.float32)
            nc.tensor.matmul(qT[:], lhsT=qf[:], rhs=ident[:], start=True, stop=True)

            # --- qry' = qry - (qry == 0): zeros become -1 ---
            eq0 = pool.tile([L, B_LOC], dt.float32)
            qp = pool.tile([L, B_LOC], dt.float32)
            nc.vector.tensor_scalar(
                out=eq0[:], in0=qT[:], scalar1=0.0, scalar2=None,
                op0=mybir.AluOpType.is_equal,
            )
            nc.vector.tensor_tensor(
                out=qp[:], in0=qT[:], in1=eq0[:], op=mybir.AluOpType.subtract,
            )
            # wait absorber: the first is_equal below needs both the bc
            # matmul (PE) and qp (DVE); this dummy read of qp advances DVE's
            # observed clock so the is_equal chain only waits on PE.
            qp_probe = pool.tile([1, 1], dt.float32)
            nc.vector.tensor_copy(out=qp_probe[:], in_=qp[:1, :1])

            # --- main loop: broadcast cnd row, compare, accumulate ---
            out_sb = pool.tile([L, B_LOC * L], dt.float32)
            for b in range(B_LOC):
                bc = psum.tile([L, L], dt.float32, tag="bc")
                # bc[q, c] = cnd[b, c] for all q  (row-selector matmul)
                nc.tensor.matmul(
                    bc[:], lhsT=bigsel[:, b, :], rhs=cf[:],
                    start=True, stop=True,
                )
                nc.vector.tensor_scalar(
                    out=out_sb[:, b * L : (b + 1) * L],
                    in0=bc[:],
                    scalar1=qp[:, b : b + 1],
                    scalar2=None,
                    op0=mybir.AluOpType.is_equal,
                )

            # --- store: 2 DMAs of 8 batches each (keep lane count low) ---
            GRP = 8
            for g in range(B_LOC // GRP):
                src = out_sb[:, g * GRP * L : (g + 1) * GRP * L].rearrange(
                    "q (b c) -> q b c", b=GRP
                )
                dst = out_d[g * GRP : (g + 1) * GRP].rearrange("b q c -> q b c")
                nc.sync.dma_start(out=dst, in_=src)

    _split_tail_waits(nc, mybir)
    nc.finalize()
    return nc


def _split_tail_waits(nc, mybir):
    """The walrus in this container rejects instructions carrying several
    sync waits ("Too many sync wait commands"). Tile's kernel-tail Drain
    aggregates one wait per live semaphore, so split all but one of them
    onto single-wait NoOps appended to the preceding block (same engine =
    SP, so sequencer program order preserves the semantics)."""
    f = nc.m.functions[0]
    blocks = f.blocks
    for bi, blk in enumerate(blocks):
        insts = blk.instructions
        for ii, ins in enumerate(insts):
            si = ins.sync_info
            if si is None or len(si.on_wait) <= 1:
                continue
            assert ii == 0 and bi > 0 and ins.engine == mybir.EngineType.SP, (
                f"multi-wait instruction {ins.name} at block {blk.name}[{ii}] "
                f"on {ins.engine}: only the tail drain is expected here"
            )
            waits = list(si.on_wait)
            prev = blocks[bi - 1]
            for k, w in enumerate(waits[:-1]):
                prev.add_instruction(
                    mybir.InstNoOp(
                        name=f"I-waitsplit-{bi}-{k}",
                        engine=ins.engine,
                        sync_info=mybir.SyncInfo(on_wait=[w], on_update=[]),
                    )
                )
            ins.sync_info = mybir.SyncInfo(
                on_wait=[waits[-1]], on_update=list(si.on_update)
            )


def _get_nc():
    if "nc" not in _CACHE:
        _CACHE["nc"] = _build_nc()
    return _CACHE["nc"]


def _pack(q, c):
    """Build per-core packed int32 inputs from full [B, L] id arrays."""
    consts = _consts_i32()
    packs = []
    for i in range(N_CORES):
        qs = q[i * B_LOC : (i + 1) * B_LOC]
        cs = c[i * B_LOC : (i + 1) * B_LOC]
        packs.append(np.ascontiguousarray(np.concatenate([qs, cs, consts], axis=1)))
    return packs


def _run(q, c, **spmd_kwargs):
    """Shard [B, L] int32 inputs over the 8 cores and run the Bass kernel.

    Returns the BassKernelResults (results per core + optional trace info).
    """
    from concourse.bass_utils import run_bass_kernel_spmd

    nc = _get_nc()
    in_maps = [{"packed": p} for p in _pack(q, c)]
    return run_bass_kernel_spmd(nc, in_maps, core_ids=list(range(N_CORES)), **spmd_kwargs)


def kernel(emb_weight=None, qry_lkup=None, cnd_lkup=None, **_ignored):
    q = np.ascontiguousarray(np.asarray(qry_lkup, dtype=np.int32))
    c = np.ascontiguousarray(np.asarray(cnd_lkup, dtype=np.int32))
    assert q.shape == (B, L) and c.shape == (B, L)

    res = _run(q, c)
    out = np.concatenate([r["out"] for r in res.results], axis=0)
    return out


# revision 14
# speedup vs baseline: 1.0824x; 1.0824x over previous
"""Trainium2 Bass kernel for nn_AlignBinary (token-equality similarity).

Reference semantics: with emb_weight fixed to the identity matrix, the
one-hot bmm + mask reduces exactly to

    out[b, q, c] = 1.0 if (qry[b,q] == cnd[b,c] and qry[b,q] > 0) else 0.0

Strategy (pure data parallel, batch B=128 split over 8 cores, 16 each):
  - host stages per-core inputs as f32 (exact for ids < 2^24): qryT
    [128, 16] (token q on partitions) and one row [1, 2176] holding the
    16 cnd rows (2048) plus a ones(128) vector.
  - device remaps qry zeros to -1 (qry' = qry - (qry==0)); a single
    is_equal(qry'[q], cnd[c]) then realizes sim * mask (a -1 never
    matches a cnd value in [0, 1023], and equal nonzero pairs imply both
    masks set).
  - PE broadcasts the cnd rows to all 128 partitions with 4 K=1
    ones-outer-product matmuls (N=512 each -> one PSUM bank per group of
    4 batches).
  - DVE compares each PSUM bank against the per-partition qry' column
    (free-dim 0-stride broadcast) -> 4 wide is_equal ops.
  - 4 output DMAs of 4 batches each overlap the compute.

Raw bass (no TileContext): the Tile entry/exit all-engine barriers cost
~6 us on a ~10 us kernel, and the manual semaphore schedule here is
simple: one DMA-in sem, a PE sem, a DVE sem, a DMA-out sem.
"""

import numpy as np

B = 128
L = 128
N_CORES = 8
B_LOC = B // N_CORES    # 16 batches per core
NG = 4                  # batch groups (one PSUM bank each)
GSZ = B_LOC // NG       # 4 batches per group
ROWW = B_LOC * L + L    # 2176: cnd rows + ones

_CACHE: dict = {}


def _build_nc():
    import concourse.bass as bass
    import concourse.mybir as mybir

    dt = mybir.dt
    nc = bass.Bass(trn_type="TRN2", name="align_binary")

    qt_d = nc.dram_tensor("qt", [L, B_LOC], dt.float32, kind="ExternalInput")
    row_d = nc.dram_tensor("row", [1, ROWW], dt.float32, kind="ExternalInput")
    out_d = nc.dram_tensor("out", [B_LOC, L, L], dt.float32, kind="ExternalOutput")

    with (
        nc.sbuf_tensor([L, B_LOC], dt.float32) as qts,
        nc.sbuf_tensor([1, ROWW], dt.float32) as rowt,
        nc.sbuf_tensor([L, B_LOC], dt.float32) as eq0,
        nc.sbuf_tensor([L, B_LOC], dt.float32) as qp,
        nc.sbuf_tensor([L, B_LOC * L], dt.float32) as out_sb,
        nc.psum_tensor([L, NG, GSZ * L], dt.float32) as bc,
        nc.semaphore() as s_in,
        nc.semaphore() as s_pe,
        nc.semaphore() as s_dv,
        nc.semaphore() as s_out,
        nc.semaphore() as s_q,
        nc.Block() as block,
    ):
        ones_ap = rowt[0:1, B_LOC * L : B_LOC * L + L]

        @block.sync
        def _(sync):
            sync.dma_start(qts[:], qt_d[:]).then_inc(s_in, 16)
            sync.dma_start(rowt[:], row_d[:]).then_inc(s_in, 16)
            for g in range(NG):
                sync.wait_ge(s_dv, g + 1)
                src = out_sb[:, g * GSZ * L : (g + 1) * GSZ * L].rearrange(
                    "q (b c) -> q b c", b=GSZ
                )
                dst = out_d[g * GSZ : (g + 1) * GSZ].rearrange("b q c -> q b c")
                sync.dma_start(dst, src).then_inc(s_out, 16)
            sync.wait_ge(s_out, 16 * NG)

        @block.tensor
        def _(tensor):
            tensor.wait_ge(s_in, 32)
            for g in range(NG):
                # bc[q, g, :] = ones[q] * cnd_rows[g*512:(g+1)*512]
                nc.tensor.matmul(
                    bc[:, g, :],
                    lhsT=ones_ap,
                    rhs=rowt[0:1, g * GSZ * L : (g + 1) * GSZ * L],
                    start=True,
                    stop=True,
                ).then_inc(s_pe, 1)

        @block.vector
        def _(vector):
            vector.wait_ge(s_in, 32)
            # qry' = qry - (qry == 0): zeros become -1. The DVE pipeline has
            # no interlocks: same-engine RAW chains need sem waits.
            nc.vector.tensor_scalar(
                out=eq0[:], in0=qts[:], scalar1=0.0, scalar2=None,
                op0=mybir.AluOpType.is_equal,
            ).then_inc(s_q, 1)
            vector.wait_ge(s_q, 1)
            nc.vector.tensor_tensor(
                out=qp[:], in0=qts[:], in1=eq0[:], op=mybir.AluOpType.subtract,
            ).then_inc(s_q, 1)
            vector.wait_ge(s_q, 2)
            for g in range(NG):
                vector.wait_ge(s_pe, g + 1)
                # out[q, b, c] = (bc[q, b, c] == qry'[q, b])  [b broadcast 128x]
                # AP steps/offsets are in elements; partition dim first.
                in1 = bass.AP(qp, g * GSZ, [[B_LOC, L], [1, GSZ], [0, L]])
                nc.vector.tensor_tensor(
                    out=out_sb[:, g * GSZ * L : (g + 1) * GSZ * L].rearrange(
                        "q (b c) -> q b c", b=GSZ
                    ),
                    in0=bc[:, g, :].rearrange("q (b c) -> q b c", b=GSZ),
                    in1=in1,
                    op=mybir.AluOpType.is_equal,
                ).then_inc(s_dv, 1)

    nc.finalize()
    return nc


def _get_nc():
    if "nc" not in _CACHE:
        _CACHE["nc"] = _build_nc()
    return _CACHE["nc"]


def _pack(q, c):
    """Stage per-core inputs: qryT f32 [L, B_LOC] and the cnd+ones row."""
    maps = []
    ones = np.ones((L,), dtype=np.float32)
    for i in range(N_CORES):
        qs = q[i * B_LOC : (i + 1) * B_LOC]
        cs = c[i * B_LOC : (i + 1) * B_LOC]
        qt = np.ascontiguousarray(qs.T.astype(np.float32))
        row = np.concatenate([cs.astype(np.float32).reshape(-1), ones])[None, :]
        maps.append({"qt": qt, "row": np.ascontiguousarray(row)})
    return maps


def _run(q, c, **spmd_kwargs):
    """Shard [B, L] inputs over the 8 cores and run the Bass kernel.

    Returns the BassKernelResults (results per core + optional trace info).
    """
    from concourse.bass_utils import run_bass_kernel_spmd

    nc = _get_nc()
    in_maps = _pack(q, c)
    return run_bass_kernel_spmd(nc, in_maps, core_ids=list(range(N_CORES)), **spmd_kwargs)


def kernel(emb_weight=None, qry_lkup=None, cnd_lkup=None, **_ignored):
    q = np.asarray(qry_lkup, dtype=np.int64)
    c = np.asarray(cnd_lkup, dtype=np.int64)
    assert q.shape == (B, L) and c.shape == (B, L)

    res = _run(q, c)
    out = np.concatenate([r["out"] for r in res.results], axis=0)
    return out


# revision 18
# speedup vs baseline: 1.2866x; 1.1887x over previous
"""Trainium2 Bass kernel for nn_AlignBinary (token-equality similarity).

Reference semantics: with emb_weight fixed to the identity matrix, the
one-hot bmm + mask reduces exactly to

    out[b, q, c] = 1.0 if (qry[b,q] == cnd[b,c] and qry[b,q] > 0) else 0.0

Strategy (pure data parallel, batch B=128 split over 8 cores, 16 each):
  - host stages per-core inputs as f32 (exact for ids < 2^24): qryT
    [128, 16] (token q on partitions) and one row [1, 2176] holding the
    16 cnd rows (2048) plus a ones(128) vector.
  - device remaps qry zeros to -1 (qry' = qry - (qry==0)); a single
    is_equal(qry'[q], cnd[c]) then realizes sim * mask (a -1 never
    matches a cnd value in [0, 1023], and equal nonzero pairs imply both
    masks set).
  - PE broadcasts the cnd rows to all 128 partitions with 4 K=1
    ones-outer-product matmuls (N=512 each -> one PSUM bank per group of
    4 batches).
  - DVE compares each PSUM bank against the per-partition qry' column
    (free-dim 0-stride broadcast) -> 4 wide is_equal ops.
  - 4 output DMAs of 4 batches each overlap the compute.

Raw bass (no TileContext): the Tile entry/exit all-engine barriers cost
~6 us on a ~10 us kernel, and the manual semaphore schedule here is
simple: one DMA-in sem, a PE sem, a DVE sem, a DMA-out sem.
"""

import numpy as np

B = 128
L = 128
N_CORES = 8
B_LOC = B // N_CORES    # 16 batches per core
NG = 4                  # batch groups (one PSUM bank each)
GSZ = B_LOC // NG       # 4 batches per group
ROWW = B_LOC * L + L    # 2176: cnd rows + ones

_CACHE: dict = {}


def _build_nc():
    import concourse.bass as bass
    import concourse.mybir as mybir

    dt = mybir.dt
    nc = bass.Bass(trn_type="TRN2", name="align_binary")

    qt_d = nc.dram_tensor("qt", [L, B_LOC], dt.float32, kind="ExternalInput")
    # fp16: ids <= 1023 are exact, and fp16 matmuls are single-pass on the
    # PE (f32 matmuls decompose into two bf16 passes, 2x LDW+MM cost).
    row_d = nc.dram_tensor("row", [1, ROWW], dt.float16, kind="ExternalInput")
    out_d = nc.dram_tensor("out", [B_LOC, L, L], dt.float32, kind="ExternalOutput")

    with (
        nc.sbuf_tensor([L, B_LOC], dt.float32) as qts,
        nc.sbuf_tensor([1, ROWW], dt.float16) as rowt,
        nc.sbuf_tensor([L, B_LOC], dt.float32) as eq0,
        nc.sbuf_tensor([L, B_LOC], dt.float32) as qp,
        nc.sbuf_tensor([L, B_LOC * L], dt.float32) as out_sb,
        nc.psum_tensor([L, NG, GSZ * L], dt.float32) as bc,
        nc.semaphore() as s_in,
        nc.semaphore() as s_inq,
        nc.semaphore() as s_pe,
        nc.semaphore() as s_dv,
        nc.semaphore() as s_out,
        nc.semaphore() as s_q,
        nc.Block(no_gpsimd_drain=True) as block,
    ):
        ones_ap = rowt[0:1, B_LOC * L : B_LOC * L + L]

        @block.sync
        def _(sync):
            sync.dma_start(rowt[:], row_d[:]).then_inc(s_in, 16)
            sync.dma_start(qts[:], qt_d[:]).then_inc(s_inq, 16)
            for g in range(NG):
                sync.wait_ge(s_dv, g + 1)
                src = out_sb[:, g * GSZ * L : (g + 1) * GSZ * L].rearrange(
                    "q (b c) -> q b c", b=GSZ
                )
                dst = out_d[g * GSZ : (g + 1) * GSZ].rearrange("b q c -> q b c")
                sync.dma_start(dst, src).then_inc(s_out, 16)
            sync.wait_ge(s_out, 16 * NG)

        @block.tensor
        def _(tensor):
            tensor.wait_ge(s_in, 16)
            for g in range(NG):
                # bc[q, g, :] = ones[q] * cnd_rows[g*512:(g+1)*512]
                nc.tensor.matmul(
                    bc[:, g, :],
                    lhsT=ones_ap,
                    rhs=rowt[0:1, g * GSZ * L : (g + 1) * GSZ * L],
                    start=True,
                    stop=True,
                ).then_inc(s_pe, 1)

        @block.vector
        def _(vector):
            vector.wait_ge(s_inq, 16)
            # qry' = qry - (qry == 0): zeros become -1. The DVE pipeline has
            # no interlocks: same-engine RAW chains need sem waits.
            nc.vector.tensor_scalar(
                out=eq0[:], in0=qts[:], scalar1=0.0, scalar2=None,
                op0=mybir.AluOpType.is_equal,
            ).then_inc(s_q, 1)
            vector.wait_ge(s_q, 1)
            nc.vector.tensor_tensor(
                out=qp[:], in0=qts[:], in1=eq0[:], op=mybir.AluOpType.subtract,
            ).then_inc(s_q, 1)
            vector.wait_ge(s_q, 2)
            for g in range(NG):
                vector.wait_ge(s_pe, g + 1)
                # out[q, b, c] = (bc[q, b, c] == qry'[q, b])  [b broadcast 128x]
                # AP steps/offsets are in elements; partition dim first.
                in1 = bass.AP(qp, g * GSZ, [[B_LOC, L], [1, GSZ], [0, L]])
                nc.vector.tensor_tensor(
                    out=out_sb[:, g * GSZ * L : (g + 1) * GSZ * L].rearrange(
                        "q (b c) -> q b c", b=GSZ
                    ),
                    in0=bc[:, g, :].rearrange("q (b c) -> q b c", b=GSZ),
                    in1=in1,
                    op=mybir.AluOpType.is_equal,
                ).then_inc(s_dv, 1)

    nc.finalize()
    return nc


def _get_nc():
    if "nc" not in _CACHE:
        _CACHE["nc"] = _build_nc()
    return _CACHE["nc"]


def _pack(q, c):
    """Stage per-core inputs: qryT f32 [L, B_LOC] and the cnd+ones row."""
    maps = []
    ones = np.ones((L,), dtype=np.float16)
    for i in range(N_CORES):
        qs = q[i * B_LOC : (i + 1) * B_LOC]
        cs = c[i * B_LOC : (i + 1) * B_LOC]
        qt = np.ascontiguousarray(qs.T.astype(np.float32))
        row = np.concatenate([cs.astype(np.float16).reshape(-1), ones])[None, :]
        maps.append({"qt": qt, "row": np.ascontiguousarray(row)})
    return maps


def _run(q, c, **spmd_kwargs):
    """Shard [B, L] inputs over the 8 cores and run the Bass kernel.

    Returns the BassKernelResults (results per core + optional trace info).
    """
    from concourse.bass_utils import run_bass_kernel_spmd

    nc = _get_nc()
    in_maps = _pack(q, c)
    return run_bass_kernel_spmd(nc, in_maps, core_ids=list(range(N_CORES)), **spmd_kwargs)


def kernel(emb_weight=None, qry_lkup=None, cnd_lkup=None, **_ignored):
    q = np.asarray(qry_lkup, dtype=np.int64)
    c = np.asarray(cnd_lkup, dtype=np.int64)
    assert q.shape == (B, L) and c.shape == (B, L)

    res = _run(q, c)
    out = np.concatenate([r["out"] for r in res.results], axis=0)
    return out


# revision 19
# speedup vs baseline: 1.3007x; 1.0110x over previous
"""Trainium2 Bass kernel for nn_AlignBinary (token-equality similarity).

Reference semantics: with emb_weight fixed to the identity matrix, the
one-hot bmm + mask reduces exactly to

    out[b, q, c] = 1.0 if (qry[b,q] == cnd[b,c] and qry[b,q] > 0) else 0.0

Strategy (pure data parallel, batch B=128 split over 8 cores, 16 each):
  - host stages per-core inputs as f32 (exact for ids < 2^24): qryT
    [128, 16] (token q on partitions) and one row [1, 2176] holding the
    16 cnd rows (2048) plus a ones(128) vector.
  - device remaps qry zeros to -1 (qry' = qry - (qry==0)); a single
    is_equal(qry'[q], cnd[c]) then realizes sim * mask (a -1 never
    matches a cnd value in [0, 1023], and equal nonzero pairs imply both
    masks set).
  - PE broadcasts the cnd rows to all 128 partitions with 4 K=1
    ones-outer-product matmuls (N=512 each -> one PSUM bank per group of
    4 batches).
  - DVE compares each PSUM bank against the per-partition qry' column
    (free-dim 0-stride broadcast) -> 4 wide is_equal ops.
  - 4 output DMAs of 4 batches each overlap the compute.

Raw bass (no TileContext): the Tile entry/exit all-engine barriers cost
~6 us on a ~10 us kernel, and the manual semaphore schedule here is
simple: one DMA-in sem, a PE sem, a DVE sem, a DMA-out sem.
"""

import numpy as np

B = 128
L = 128
N_CORES = 8
B_LOC = B // N_CORES    # 16 batches per core
NG = 4                  # batch groups (one PSUM bank each)
GSZ = B_LOC // NG       # 4 batches per group
ROWW = B_LOC * L + L    # 2176: cnd rows + ones

_CACHE: dict = {}


def _build_nc():
    import concourse.bass as bass
    import concourse.mybir as mybir

    dt = mybir.dt
    nc = bass.Bass(trn_type="TRN2", name="align_binary")

    qt_d = nc.dram_tensor("qt", [L, B_LOC], dt.float32, kind="ExternalInput")
    # fp16: ids <= 1023 are exact, and fp16 matmuls are single-pass on the
    # PE (f32 matmuls decompose into two bf16 passes, 2x LDW+MM cost).
    row_d = nc.dram_tensor("row", [1, ROWW], dt.float16, kind="ExternalInput")
    out_d = nc.dram_tensor("out", [B_LOC, L, L], dt.float32, kind="ExternalOutput")

    with (
        nc.sbuf_tensor([L, B_LOC], dt.float32) as qts,
        nc.sbuf_tensor([1, ROWW], dt.float16) as rowt,
        nc.sbuf_tensor([L, B_LOC], dt.float32) as eq0,
        nc.sbuf_tensor([L, B_LOC], dt.float32) as qp,
        nc.sbuf_tensor([L, B_LOC * L], dt.float32) as out_sb,
        nc.psum_tensor([L, NG, GSZ * L], dt.float32) as bc,
        nc.semaphore() as s_in,
        nc.semaphore() as s_inq,
        nc.semaphore() as s_pe,
        nc.semaphore() as s_dv,
        nc.semaphore() as s_out,
        nc.semaphore() as s_q,
        nc.Block(no_gpsimd_drain=True) as block,
    ):
        ones_ap = rowt[0:1, B_LOC * L : B_LOC * L + L]

        H = GSZ // 2  # half-group: 2 batches per output DMA

        def _out_dma(eng, g, h):
            lo = (g * GSZ + h * H) * L
            src = out_sb[:, lo : lo + H * L].rearrange("q (b c) -> q b c", b=H)
            dst = out_d[g * GSZ + h * H : g * GSZ + (h + 1) * H].rearrange(
                "b q c -> q b c"
            )
            eng.dma_start(dst, src).then_inc(s_out, 16)

        @block.sync
        def _(sync):
            sync.dma_start(rowt[:], row_d[:]).then_inc(s_in, 16)
            sync.dma_start(qts[:], qt_d[:]).then_inc(s_inq, 16)
            for g in range(NG):
                sync.wait_ge(s_dv, g + 1)
                _out_dma(sync, g, 0)
            sync.wait_ge(s_out, 16 * 2 * NG)

        @block.scalar
        def _(scalar):
            for g in range(NG):
                scalar.wait_ge(s_dv, g + 1)
                _out_dma(scalar, g, 1)

        @block.tensor
        def _(tensor):
            tensor.wait_ge(s_in, 16)
            for g in range(NG):
                # bc[q, g, :] = ones[q] * cnd_rows[g*512:(g+1)*512]
                nc.tensor.matmul(
                    bc[:, g, :],
                    lhsT=ones_ap,
                    rhs=rowt[0:1, g * GSZ * L : (g + 1) * GSZ * L],
                    start=True,
                    stop=True,
                ).then_inc(s_pe, 1)

        @block.vector
        def _(vector):
            vector.wait_ge(s_inq, 16)
            # qry' = qry - (qry == 0): zeros become -1. The DVE pipeline has
            # no interlocks: same-engine RAW chains need sem waits.
            nc.vector.tensor_scalar(
                out=eq0[:], in0=qts[:], scalar1=0.0, scalar2=None,
                op0=mybir.AluOpType.is_equal,
            ).then_inc(s_q, 1)
            vector.wait_ge(s_q, 1)
            nc.vector.tensor_tensor(
                out=qp[:], in0=qts[:], in1=eq0[:], op=mybir.AluOpType.subtract,
            ).then_inc(s_q, 1)
            vector.wait_ge(s_q, 2)
            for g in range(NG):
                vector.wait_ge(s_pe, g + 1)
                # out[q, b, c] = (bc[q, b, c] == qry'[q, b])  [b broadcast 128x]
                # AP steps/offsets are in elements; partition dim first.
                in1 = bass.AP(qp, g * GSZ, [[B_LOC, L], [1, GSZ], [0, L]])
                nc.vector.tensor_tensor(
                    out=out_sb[:, g * GSZ * L : (g + 1) * GSZ * L].rearrange(
                        "q (b c) -> q b c", b=GSZ
                    ),
                    in0=bc[:, g, :].rearrange("q (b c) -> q b c", b=GSZ),
                    in1=in1,
                    op=mybir.AluOpType.is_equal,
                ).then_inc(s_dv, 1)

    nc.finalize()
    return nc


def _get_nc():
    if "nc" not in _CACHE:
        _CACHE["nc"] = _build_nc()
    return _CACHE["nc"]


def _pack(q, c):
    """Stage per-core inputs: qryT f32 [L, B_LOC] and the cnd+ones row."""
    maps = []
    ones = np.ones((L,), dtype=np.float16)
    for i in range(N_CORES):
        qs = q[i * B_LOC : (i + 1) * B_LOC]
        cs = c[i * B_LOC : (i + 1) * B_LOC]
        qt = np.ascontiguousarray(qs.T.astype(np.float32))
        row = np.concatenate([cs.astype(np.float16).reshape(-1), ones])[None, :]
        maps.append({"qt": qt, "row": np.ascontiguousarray(row)})
    return maps


def _run(q, c, **spmd_kwargs):
    """Shard [B, L] inputs over the 8 cores and run the Bass kernel.

    Returns the BassKernelResults (results per core + optional trace info).
    """
    from concourse.bass_utils import run_bass_kernel_spmd

    nc = _get_nc()
    in_maps = _pack(q, c)
    return run_bass_kernel_spmd(nc, in_maps, core_ids=list(range(N_CORES)), **spmd_kwargs)


def kernel(emb_weight=None, qry_lkup=None, cnd_lkup=None, **_ignored):
    q = np.asarray(qry_lkup, dtype=np.int64)
    c = np.asarray(cnd_lkup, dtype=np.int64)
    assert q.shape == (B, L) and c.shape == (B, L)

    res = _run(q, c)
    out = np.concatenate([r["out"] for r in res.results], axis=0)
    return out


# revision 20
# speedup vs baseline: 1.3858x; 1.0654x over previous
"""Trainium2 Bass kernel for nn_AlignBinary (token-equality similarity).

Reference semantics: with emb_weight fixed to the identity matrix, the
one-hot bmm + mask reduces exactly to

    out[b, q, c] = 1.0 if (qry[b,q] == cnd[b,c] and qry[b,q] > 0) else 0.0

Strategy (pure data parallel, batch B=128 split over 8 cores, 16 each):
  - host stages per-core inputs as f32 (exact for ids < 2^24): qryT
    [128, 16] (token q on partitions) and one row [1, 2176] holding the
    16 cnd rows (2048) plus a ones(128) vector.
  - device remaps qry zeros to -1 (qry' = qry - (qry==0)); a single
    is_equal(qry'[q], cnd[c]) then realizes sim * mask (a -1 never
    matches a cnd value in [0, 1023], and equal nonzero pairs imply both
    masks set).
  - PE broadcasts the cnd rows to all 128 partitions with 4 K=1
    ones-outer-product matmuls (N=512 each -> one PSUM bank per group of
    4 batches).
  - DVE compares each PSUM bank against the per-partition qry' column
    (free-dim 0-stride broadcast) -> 4 wide is_equal ops.
  - 4 output DMAs of 4 batches each overlap the compute.

Raw bass (no TileContext): the Tile entry/exit all-engine barriers cost
~6 us on a ~10 us kernel, and the manual semaphore schedule here is
simple: one DMA-in sem, a PE sem, a DVE sem, a DMA-out sem.
"""

import numpy as np

B = 128
L = 128
N_CORES = 8
B_LOC = B // N_CORES    # 16 batches per core
NG = 4                  # batch groups (one PSUM bank each)
GSZ = B_LOC // NG       # 4 batches per group
ROWW = B_LOC * L + L    # 2176: cnd rows + ones

_CACHE: dict = {}


def _build_nc():
    import concourse.bass as bass
    import concourse.mybir as mybir

    dt = mybir.dt
    nc = bass.Bass(trn_type="TRN2", name="align_binary")

    qt_d = nc.dram_tensor("qt", [L, B_LOC], dt.float32, kind="ExternalInput")
    # fp16: ids <= 1023 are exact, and fp16 matmuls are single-pass on the
    # PE (f32 matmuls decompose into two bf16 passes, 2x LDW+MM cost).
    row_d = nc.dram_tensor("row", [1, ROWW], dt.float16, kind="ExternalInput")
    out_d = nc.dram_tensor("out", [B_LOC, L, L], dt.float32, kind="ExternalOutput")

    with (
        nc.sbuf_tensor([L, B_LOC], dt.float32) as qts,
        nc.sbuf_tensor([1, ROWW], dt.float16) as rowt,
        nc.sbuf_tensor([L, B_LOC], dt.float32) as eq0,
        nc.sbuf_tensor([L, B_LOC], dt.float32) as qp,
        nc.sbuf_tensor([L, B_LOC * L], dt.float32) as out_sb,
        nc.psum_tensor([L, NG, GSZ * L], dt.float32) as bc,
        nc.semaphore() as s_in,
        nc.semaphore() as s_inq,
        nc.semaphore() as s_pe,
        nc.semaphore() as s_dv,
        nc.semaphore() as s_out,
        nc.semaphore() as s_q,
        nc.Block(no_gpsimd_drain=True) as block,
    ):
        ones_ap = rowt[0:1, B_LOC * L : B_LOC * L + L]

        H = GSZ // 2  # half-group: 2 batches per output DMA

        def _out_dma(eng, g, h):
            lo = (g * GSZ + h * H) * L
            src = out_sb[:, lo : lo + H * L].rearrange("q (b c) -> q b c", b=H)
            dst = out_d[g * GSZ + h * H : g * GSZ + (h + 1) * H].rearrange(
                "b q c -> q b c"
            )
            eng.dma_start(dst, src).then_inc(s_out, 16)

        @block.sync
        def _(sync):
            sync.dma_start(rowt[:], row_d[:]).then_inc(s_in, 16)
            sync.dma_start(qts[:], qt_d[:]).then_inc(s_inq, 16)
            for g in range(NG):
                sync.wait_ge(s_dv, g + 1)
                _out_dma(sync, g, 0)
            sync.wait_ge(s_out, 16 * 2 * NG)

        @block.scalar
        def _(scalar):
            for g in range(NG):
                scalar.wait_ge(s_dv, g + 1)
                _out_dma(scalar, g, 1)

        @block.tensor
        def _(tensor):
            tensor.wait_ge(s_in, 16)
            for g in range(NG):
                # bc[q, g, :] = ones[q] * cnd_rows[g*512:(g+1)*512]
                nc.tensor.matmul(
                    bc[:, g, :],
                    lhsT=ones_ap,
                    rhs=rowt[0:1, g * GSZ * L : (g + 1) * GSZ * L],
                    start=True,
                    stop=True,
                ).then_inc(s_pe, 1)

        @block.vector
        def _(vector):
            vector.wait_ge(s_inq, 16)
            # qry' = qry - (qry == 0): zeros become -1. The DVE pipeline has
            # no interlocks: same-engine RAW chains need sem waits.
            nc.vector.tensor_scalar(
                out=eq0[:], in0=qts[:], scalar1=0.0, scalar2=None,
                op0=mybir.AluOpType.is_equal,
            ).then_inc(s_q, 1)
            vector.wait_ge(s_q, 1)
            nc.vector.tensor_tensor(
                out=qp[:], in0=qts[:], in1=eq0[:], op=mybir.AluOpType.subtract,
            ).then_inc(s_q, 1)
            vector.wait_ge(s_q, 2)
            for g in range(NG):
                vector.wait_ge(s_pe, g + 1)
                # out[q, b, c] = (bc[q, b, c] == qry'[q, b])  [b broadcast 128x]
                # AP steps/offsets are in elements; partition dim first.
                in1 = bass.AP(qp, g * GSZ, [[B_LOC, L], [1, GSZ], [0, L]])
                nc.vector.tensor_tensor(
                    out=out_sb[:, g * GSZ * L : (g + 1) * GSZ * L].rearrange(
                        "q (b c) -> q b c", b=GSZ
                    ),
                    in0=bc[:, g, :].rearrange("q (b c) -> q b c", b=GSZ),
                    in1=in1,
                    op=mybir.AluOpType.is_equal,
                ).then_inc(s_dv, 1)

    _strip_barriers(nc, mybir)
    nc.finalize()
    return nc


def _strip_barriers(nc, mybir):
    """Remove bass's const-ap memsets and the entry/exit all-engine
    barriers (~2 us of exec window). All cross-engine ordering in this
    kernel flows through explicit semaphores; the runtime zero-inits
    semaphores at NEFF load, and SP only halts after s_out confirms the
    output DMAs landed, so neither barrier is load-bearing here."""
    f = nc.m.functions[0]
    drop = ("Memset", "Drain", "EventSemaphore")
    for bi, blk in enumerate(f.blocks):
        if blk.name != "main" and not blk.name.endswith("_end"):
            continue
        keep = [i for i in blk.instructions if i.opcode not in drop]
        if len(keep) != len(blk.instructions):
            f.blocks[bi] = mybir.BasicBlock(name=blk.name, instructions=keep)


def _get_nc():
    if "nc" not in _CACHE:
        _CACHE["nc"] = _build_nc()
    return _CACHE["nc"]


def _pack(q, c):
    """Stage per-core inputs: qryT f32 [L, B_LOC] and the cnd+ones row."""
    maps = []
    ones = np.ones((L,), dtype=np.float16)
    for i in range(N_CORES):
        qs = q[i * B_LOC : (i + 1) * B_LOC]
        cs = c[i * B_LOC : (i + 1) * B_LOC]
        qt = np.ascontiguousarray(qs.T.astype(np.float32))
        row = np.concatenate([cs.astype(np.float16).reshape(-1), ones])[None, :]
        maps.append({"qt": qt, "row": np.ascontiguousarray(row)})
    return maps


def _run(q, c, **spmd_kwargs):
    """Shard [B, L] inputs over the 8 cores and run the Bass kernel.

    Returns the BassKernelResults (results per core + optional trace info).
    """
    from concourse.bass_utils import run_bass_kernel_spmd

    nc = _get_nc()
    in_maps = _pack(q, c)
    return run_bass_kernel_spmd(nc, in_maps, core_ids=list(range(N_CORES)), **spmd_kwargs)


def kernel(emb_weight=None, qry_lkup=None, cnd_lkup=None, **_ignored):
    q = np.asarray(qry_lkup, dtype=np.int64)
    c = np.asarray(cnd_lkup, dtype=np.int64)
    assert q.shape == (B, L) and c.shape == (B, L)

    res = _run(q, c)
    out = np.concatenate([r["out"] for r in res.results], axis=0)
    return out


# revision 21
# speedup vs baseline: 1.6645x; 1.2011x over previous
"""Trainium2 Bass kernel for nn_AlignBinary (token-equality similarity).

Reference semantics: with emb_weight fixed to the identity matrix, the
one-hot bmm + mask reduces exactly to

    out[b, q, c] = 1.0 if (qry[b,q] == cnd[b,c] and qry[b,q] > 0) else 0.0

Strategy (pure data parallel, batch B=128 split over 8 cores, 16 each):
  - host stages per-core inputs as f32 (exact for ids < 2^24): qryT
    [128, 16] (token q on partitions) and one row [1, 2176] holding the
    16 cnd rows (2048) plus a ones(128) vector.
  - device remaps qry zeros to -1 (qry' = qry - (qry==0)); a single
    is_equal(qry'[q], cnd[c]) then realizes sim * mask (a -1 never
    matches a cnd value in [0, 1023], and equal nonzero pairs imply both
    masks set).
  - PE broadcasts the cnd rows to all 128 partitions with 4 K=1
    ones-outer-product matmuls (N=512 each -> one PSUM bank per group of
    4 batches).
  - DVE compares each PSUM bank against the per-partition qry' column
    (free-dim 0-stride broadcast) -> 4 wide is_equal ops.
  - 4 output DMAs of 4 batches each overlap the compute.

Raw bass (no TileContext): the Tile entry/exit all-engine barriers cost
~6 us on a ~10 us kernel, and the manual semaphore schedule here is
simple: one DMA-in sem, a PE sem, a DVE sem, a DMA-out sem.
"""

import numpy as np

B = 128
L = 128
N_CORES = 8
B_LOC = B // N_CORES    # 16 batches per core
NG = 4                  # batch groups (one PSUM bank each)
GSZ = B_LOC // NG       # 4 batches per group
ROWW = B_LOC * L + L    # 2176: cnd rows + ones

_CACHE: dict = {}


def _build_nc():
    import concourse.bass as bass
    import concourse.mybir as mybir

    dt = mybir.dt
    nc = bass.Bass(trn_type="TRN2", name="align_binary")

    qt_d = nc.dram_tensor("qt", [L, B_LOC], dt.float32, kind="ExternalInput")
    # fp16: ids <= 1023 are exact, and fp16 matmuls are single-pass on the
    # PE (f32 matmuls decompose into two bf16 passes, 2x LDW+MM cost).
    row_d = nc.dram_tensor("row", [1, ROWW], dt.float16, kind="ExternalInput")
    out_d = nc.dram_tensor("out", [B_LOC, L, L], dt.float32, kind="ExternalOutput")

    with (
        nc.sbuf_tensor([L, B_LOC], dt.float32) as qts,
        nc.sbuf_tensor([1, ROWW], dt.float16) as rowt,
        nc.sbuf_tensor([L, B_LOC], dt.float32) as eq0,
        nc.sbuf_tensor([L, B_LOC], dt.float32) as qp,
        nc.sbuf_tensor([L, B_LOC * L], dt.float32) as out_sb,
        nc.psum_tensor([L, NG, GSZ * L], dt.float32) as bc,
        nc.semaphore() as s_in,
        nc.semaphore() as s_inq,
        nc.semaphore() as s_pe,
        nc.semaphore() as s_dv,
        nc.semaphore() as s_out,
        nc.semaphore() as s_q,
        nc.Block(no_gpsimd_drain=True) as block,
    ):
        ones_ap = rowt[0:1, B_LOC * L : B_LOC * L + L]

        H = GSZ // 2  # half-group: 2 batches per output DMA

        def _out_dma(eng, g, h):
            lo = (g * GSZ + h * H) * L
            src = out_sb[:, lo : lo + H * L].rearrange("q (b c) -> q b c", b=H)
            dst = out_d[g * GSZ + h * H : g * GSZ + (h + 1) * H].rearrange(
                "b q c -> q b c"
            )
            eng.dma_start(dst, src).then_inc(s_out, 16)

        @block.sync
        def _(sync):
            sync.dma_start(rowt[:], row_d[:]).then_inc(s_in, 16)
            for g in range(NG):
                sync.wait_ge(s_dv, g + 1)
                _out_dma(sync, g, 0)
            sync.wait_ge(s_out, 16 * 2 * NG)

        @block.scalar
        def _(scalar):
            # dispatch qt in parallel with row (both HWDGE engines) so the
            # DVE prep chain isn't gated on a serialized second dispatch
            scalar.dma_start(qts[:], qt_d[:]).then_inc(s_inq, 16)
            for g in range(NG):
                scalar.wait_ge(s_dv, g + 1)
                _out_dma(scalar, g, 1)

        @block.tensor
        def _(tensor):
            tensor.wait_ge(s_in, 16)
            for g in range(NG):
                # bc[q, g, :] = ones[q] * cnd_rows[g*512:(g+1)*512]
                nc.tensor.matmul(
                    bc[:, g, :],
                    lhsT=ones_ap,
                    rhs=rowt[0:1, g * GSZ * L : (g + 1) * GSZ * L],
                    start=True,
                    stop=True,
                ).then_inc(s_pe, 1)

        @block.vector
        def _(vector):
            vector.wait_ge(s_inq, 16)
            # qry' = qry - (qry == 0): zeros become -1. The DVE pipeline has
            # no interlocks: same-engine RAW chains need sem waits.
            nc.vector.tensor_scalar(
                out=eq0[:], in0=qts[:], scalar1=0.0, scalar2=None,
                op0=mybir.AluOpType.is_equal,
            ).then_inc(s_q, 1)
            vector.wait_ge(s_q, 1)
            nc.vector.tensor_tensor(
                out=qp[:], in0=qts[:], in1=eq0[:], op=mybir.AluOpType.subtract,
            ).then_inc(s_q, 1)
            vector.wait_ge(s_q, 2)
            for g in range(NG):
                vector.wait_ge(s_pe, g + 1)
                # out[q, b, c] = (bc[q, b, c] == qry'[q, b])  [b broadcast 128x]
                # AP steps/offsets are in elements; partition dim first.
                in1 = bass.AP(qp, g * GSZ, [[B_LOC, L], [1, GSZ], [0, L]])
                nc.vector.tensor_tensor(
                    out=out_sb[:, g * GSZ * L : (g + 1) * GSZ * L].rearrange(
                        "q (b c) -> q b c", b=GSZ
                    ),
                    in0=bc[:, g, :].rearrange("q (b c) -> q b c", b=GSZ),
                    in1=in1,
                    op=mybir.AluOpType.is_equal,
                ).then_inc(s_dv, 1)

    _strip_barriers(nc, mybir)
    nc.finalize()
    return nc


def _strip_barriers(nc, mybir):
    """Remove bass's const-ap memsets and the entry/exit all-engine
    barriers (~2 us of exec window). All cross-engine ordering in this
    kernel flows through explicit semaphores; the runtime zero-inits
    semaphores at NEFF load, and SP only halts after s_out confirms the
    output DMAs landed, so neither barrier is load-bearing here."""
    f = nc.m.functions[0]
    drop = ("Memset", "Drain", "EventSemaphore")
    for bi, blk in enumerate(f.blocks):
        if blk.name != "main" and not blk.name.endswith("_end"):
            continue
        keep = [i for i in blk.instructions if i.opcode not in drop]
        if len(keep) != len(blk.instructions):
            f.blocks[bi] = mybir.BasicBlock(name=blk.name, instructions=keep)


def _get_nc():
    if "nc" not in _CACHE:
        _CACHE["nc"] = _build_nc()
    return _CACHE["nc"]


def _pack(q, c):
    """Stage per-core inputs: qryT f32 [L, B_LOC] and the cnd+ones row."""
    maps = []
    ones = np.ones((L,), dtype=np.float16)
    for i in range(N_CORES):
        qs = q[i * B_LOC : (i + 1) * B_LOC]
        cs = c[i * B_LOC : (i + 1) * B_LOC]
        qt = np.ascontiguousarray(qs.T.astype(np.float32))
        row = np.concatenate([cs.astype(np.float16).reshape(-1), ones])[None, :]
        maps.append({"qt": qt, "row": np.ascontiguousarray(row)})
    return maps


def _run(q, c, **spmd_kwargs):
    """Shard [B, L] inputs over the 8 cores and run the Bass kernel.

    Returns the BassKernelResults (results per core + optional trace info).
    """
    from concourse.bass_utils import run_bass_kernel_spmd

    nc = _get_nc()
    in_maps = _pack(q, c)
    return run_bass_kernel_spmd(nc, in_maps, core_ids=list(range(N_CORES)), **spmd_kwargs)


def kernel(emb_weight=None, qry_lkup=None, cnd_lkup=None, **_ignored):
    q = np.asarray(qry_lkup, dtype=np.int64)
    c = np.asarray(cnd_lkup, dtype=np.int64)
    assert q.shape == (B, L) and c.shape == (B, L)

    res = _run(q, c)
    out = np.concatenate([r["out"] for r in res.results], axis=0)
    return out


# revision 22
# speedup vs baseline: 1.6839x; 1.0117x over previous
"""Trainium2 Bass kernel for nn_AlignBinary (token-equality similarity).

Reference semantics: with emb_weight fixed to the identity matrix, the
one-hot bmm + mask reduces exactly to

    out[b, q, c] = 1.0 if (qry[b,q] == cnd[b,c] and qry[b,q] > 0) else 0.0

Strategy (pure data parallel, batch B=128 split over 8 cores, 16 each):
  - host stages per-core inputs as f32 (exact for ids < 2^24): qryT
    [128, 16] (token q on partitions) and one row [1, 2176] holding the
    16 cnd rows (2048) plus a ones(128) vector.
  - device remaps qry zeros to -1 (qry' = qry - (qry==0)); a single
    is_equal(qry'[q], cnd[c]) then realizes sim * mask (a -1 never
    matches a cnd value in [0, 1023], and equal nonzero pairs imply both
    masks set).
  - PE broadcasts the cnd rows to all 128 partitions with 4 K=1
    ones-outer-product matmuls (N=512 each -> one PSUM bank per group of
    4 batches).
  - DVE compares each PSUM bank against the per-partition qry' column
    (free-dim 0-stride broadcast) -> 4 wide is_equal ops.
  - 4 output DMAs of 4 batches each overlap the compute.

Raw bass (no TileContext): the Tile entry/exit all-engine barriers cost
~6 us on a ~10 us kernel, and the manual semaphore schedule here is
simple: one DMA-in sem, a PE sem, a DVE sem, a DMA-out sem.
"""

import numpy as np

B = 128
L = 128
N_CORES = 8
B_LOC = B // N_CORES    # 16 batches per core
NG = 4                  # batch groups (one PSUM bank each)
GSZ = B_LOC // NG       # 4 batches per group
ROWW = B_LOC * L + L    # 2176: cnd rows + ones

_CACHE: dict = {}


def _build_nc():
    import concourse.bass as bass
    import concourse.mybir as mybir

    dt = mybir.dt
    nc = bass.Bass(trn_type="TRN2", name="align_binary")

    qt_d = nc.dram_tensor("qt", [L, B_LOC], dt.float32, kind="ExternalInput")
    # fp16: ids <= 1023 are exact, and fp16 matmuls are single-pass on the
    # PE (f32 matmuls decompose into two bf16 passes, 2x LDW+MM cost).
    row_d = nc.dram_tensor("row", [1, ROWW], dt.float16, kind="ExternalInput")
    out_d = nc.dram_tensor("out", [B_LOC, L, L], dt.float32, kind="ExternalOutput")

    with (
        nc.sbuf_tensor([L, B_LOC], dt.float32) as qts,
        nc.sbuf_tensor([1, ROWW], dt.float16) as rowt,
        nc.sbuf_tensor([L, B_LOC], dt.float32) as eq0,
        nc.sbuf_tensor([L, B_LOC], dt.float32) as qp,
        nc.sbuf_tensor([L, B_LOC * L], dt.float32) as out_sb,
        nc.psum_tensor([L, NG, GSZ * L], dt.float32) as bc,
        nc.semaphore() as s_in,
        nc.semaphore() as s_inq,
        nc.semaphore() as s_pe,
        nc.semaphore() as s_dv,
        nc.semaphore() as s_out,
        nc.semaphore() as s_q,
        nc.Block(no_gpsimd_drain=True) as block,
    ):
        ones_ap = rowt[0:1, B_LOC * L : B_LOC * L + L]

        def _out_dma(eng, b0, nb):
            src = out_sb[:, b0 * L : (b0 + nb) * L].rearrange(
                "q (b c) -> q b c", b=nb
            )
            dst = out_d[b0 : b0 + nb].rearrange("b q c -> q b c")
            eng.dma_start(dst, src).then_inc(s_out, 16)

        # one DMA per group alternating dispatch engines (dispatch is
        # ~0.65us serial per engine); the last group is split across both
        # engines so its data starts moving immediately after the final eq.
        N_OUT_DMA = NG + 1

        @block.sync
        def _(sync):
            sync.dma_start(rowt[:], row_d[:]).then_inc(s_in, 16)
            sync.wait_ge(s_dv, 1)
            _out_dma(sync, 0, GSZ)
            sync.wait_ge(s_dv, 3)
            _out_dma(sync, 2 * GSZ, GSZ)
            sync.wait_ge(s_dv, 4)
            _out_dma(sync, 3 * GSZ, GSZ // 2)
            sync.wait_ge(s_out, 16 * N_OUT_DMA)

        @block.scalar
        def _(scalar):
            # dispatch qt in parallel with row (both HWDGE engines) so the
            # DVE prep chain isn't gated on a serialized second dispatch
            scalar.dma_start(qts[:], qt_d[:]).then_inc(s_inq, 16)
            scalar.wait_ge(s_dv, 2)
            _out_dma(scalar, GSZ, GSZ)
            scalar.wait_ge(s_dv, 4)
            _out_dma(scalar, 3 * GSZ + GSZ // 2, GSZ // 2)

        @block.tensor
        def _(tensor):
            tensor.wait_ge(s_in, 16)
            for g in range(NG):
                # bc[q, g, :] = ones[q] * cnd_rows[g*512:(g+1)*512]
                nc.tensor.matmul(
                    bc[:, g, :],
                    lhsT=ones_ap,
                    rhs=rowt[0:1, g * GSZ * L : (g + 1) * GSZ * L],
                    start=True,
                    stop=True,
                ).then_inc(s_pe, 1)

        @block.vector
        def _(vector):
            vector.wait_ge(s_inq, 16)
            # qry' = qry - (qry == 0): zeros become -1. The DVE pipeline has
            # no interlocks: same-engine RAW chains need sem waits.
            nc.vector.tensor_scalar(
                out=eq0[:], in0=qts[:], scalar1=0.0, scalar2=None,
                op0=mybir.AluOpType.is_equal,
            ).then_inc(s_q, 1)
            vector.wait_ge(s_q, 1)
            nc.vector.tensor_tensor(
                out=qp[:], in0=qts[:], in1=eq0[:], op=mybir.AluOpType.subtract,
            ).then_inc(s_q, 1)
            vector.wait_ge(s_q, 2)
            for g in range(NG):
                vector.wait_ge(s_pe, g + 1)
                # out[q, b, c] = (bc[q, b, c] == qry'[q, b])  [b broadcast 128x]
                # AP steps/offsets are in elements; partition dim first.
                in1 = bass.AP(qp, g * GSZ, [[B_LOC, L], [1, GSZ], [0, L]])
                nc.vector.tensor_tensor(
                    out=out_sb[:, g * GSZ * L : (g + 1) * GSZ * L].rearrange(
                        "q (b c) -> q b c", b=GSZ
                    ),
                    in0=bc[:, g, :].rearrange("q (b c) -> q b c", b=GSZ),
                    in1=in1,
                    op=mybir.AluOpType.is_equal,
                ).then_inc(s_dv, 1)

    _strip_barriers(nc, mybir)
    nc.finalize()
    return nc


def _strip_barriers(nc, mybir):
    """Remove bass's const-ap memsets and the entry/exit all-engine
    barriers (~2 us of exec window). All cross-engine ordering in this
    kernel flows through explicit semaphores; the runtime zero-inits
    semaphores at NEFF load, and SP only halts after s_out confirms the
    output DMAs landed, so neither barrier is load-bearing here."""
    f = nc.m.functions[0]
    drop = ("Memset", "Drain", "EventSemaphore")
    for bi, blk in enumerate(f.blocks):
        if blk.name != "main" and not blk.name.endswith("_end"):
            continue
        keep = [i for i in blk.instructions if i.opcode not in drop]
        if len(keep) != len(blk.instructions):
            f.blocks[bi] = mybir.BasicBlock(name=blk.name, instructions=keep)


def _get_nc():
    if "nc" not in _CACHE:
        _CACHE["nc"] = _build_nc()
    return _CACHE["nc"]


def _pack(q, c):
    """Stage per-core inputs: qryT f32 [L, B_LOC] and the cnd+ones row."""
    maps = []
    ones = np.ones((L,), dtype=np.float16)
    for i in range(N_CORES):
        qs = q[i * B_LOC : (i + 1) * B_LOC]
        cs = c[i * B_LOC : (i + 1) * B_LOC]
        qt = np.ascontiguousarray(qs.T.astype(np.float32))
        row = np.concatenate([cs.astype(np.float16).reshape(-1), ones])[None, :]
        maps.append({"qt": qt, "row": np.ascontiguousarray(row)})
    return maps


def _run(q, c, **spmd_kwargs):
    """Shard [B, L] inputs over the 8 cores and run the Bass kernel.

    Returns the BassKernelResults (results per core + optional trace info).
    """
    from concourse.bass_utils import run_bass_kernel_spmd

    nc = _get_nc()
    in_maps = _pack(q, c)
    return run_bass_kernel_spmd(nc, in_maps, core_ids=list(range(N_CORES)), **spmd_kwargs)


def kernel(emb_weight=None, qry_lkup=None, cnd_lkup=None, **_ignored):
    q = np.asarray(qry_lkup, dtype=np.int64)
    c = np.asarray(cnd_lkup, dtype=np.int64)
    assert q.shape == (B, L) and c.shape == (B, L)

    res = _run(q, c)
    out = np.concatenate([r["out"] for r in res.results], axis=0)
    return out


# revision 24
# speedup vs baseline: 1.7098x; 1.0154x over previous
"""Trainium2 Bass kernel for nn_AlignBinary (token-equality similarity).

Reference semantics: with emb_weight fixed to the identity matrix, the
one-hot bmm + mask reduces exactly to

    out[b, q, c] = 1.0 if (qry[b,q] == cnd[b,c] and qry[b,q] > 0) else 0.0

Strategy (pure data parallel, batch B=128 split over 8 cores, 16 each):
  - host stages per-core inputs as f32 (exact for ids < 2^24): qryT
    [128, 16] (token q on partitions) and one row [1, 2176] holding the
    16 cnd rows (2048) plus a ones(128) vector.
  - device remaps qry zeros to -1 (qry' = qry - (qry==0)); a single
    is_equal(qry'[q], cnd[c]) then realizes sim * mask (a -1 never
    matches a cnd value in [0, 1023], and equal nonzero pairs imply both
    masks set).
  - PE broadcasts the cnd rows to all 128 partitions with 4 K=1
    ones-outer-product matmuls (N=512 each -> one PSUM bank per group of
    4 batches).
  - DVE compares each PSUM bank against the per-partition qry' column
    (free-dim 0-stride broadcast) -> 4 wide is_equal ops.
  - 4 output DMAs of 4 batches each overlap the compute.

Raw bass (no TileContext): the Tile entry/exit all-engine barriers cost
~6 us on a ~10 us kernel, and the manual semaphore schedule here is
simple: one DMA-in sem, a PE sem, a DVE sem, a DMA-out sem.
"""

import numpy as np

B = 128
L = 128
N_CORES = 8
B_LOC = B // N_CORES    # 16 batches per core
NG = 4                  # batch groups (one PSUM bank each)
GSZ = B_LOC // NG       # 4 batches per group
ROWW = B_LOC * L + L    # 2176: cnd rows + ones

_CACHE: dict = {}


def _build_nc():
    import concourse.bass as bass
    import concourse.mybir as mybir

    dt = mybir.dt
    nc = bass.Bass(trn_type="TRN2", name="align_binary")

    qt_d = nc.dram_tensor("qt", [L, B_LOC], dt.float32, kind="ExternalInput")
    # fp16: ids <= 1023 are exact, and fp16 matmuls are single-pass on the
    # PE (f32 matmuls decompose into two bf16 passes, 2x LDW+MM cost).
    row_d = nc.dram_tensor("row", [1, ROWW], dt.float16, kind="ExternalInput")
    out_d = nc.dram_tensor("out", [B_LOC, L, L], dt.float32, kind="ExternalOutput")

    with (
        nc.sbuf_tensor([L, B_LOC], dt.float32) as qts,
        nc.sbuf_tensor([1, ROWW], dt.float16) as rowt,
        nc.sbuf_tensor([L, B_LOC], dt.float32) as eq0,
        nc.sbuf_tensor([L, B_LOC], dt.float32) as qp,
        nc.sbuf_tensor([L, B_LOC * L], dt.float32) as out_sb,
        nc.psum_tensor([L, NG, GSZ * L], dt.float32) as bc,
        nc.semaphore() as s_in,
        nc.semaphore() as s_inq,
        nc.semaphore() as s_pe,
        nc.semaphore() as s_dv,
        nc.semaphore() as s_out,
        nc.semaphore() as s_q,
    ):
        # No nc.Block(): the kernel is branch-free, so every instruction is
        # emitted straight into the main basic block (each engine executes
        # its own subsequence). This drops the per-engine body branches,
        # the empty end block, and the Block exit barrier entirely.
        ones_ap = rowt[0:1, B_LOC * L : B_LOC * L + L]

        def _out_dma(eng, b0, nb):
            src = out_sb[:, b0 * L : (b0 + nb) * L].rearrange(
                "q (b c) -> q b c", b=nb
            )
            dst = out_d[b0 : b0 + nb].rearrange("b q c -> q b c")
            eng.dma_start(dst, src).then_inc(s_out, 16)

        # one DMA per group alternating dispatch engines (dispatch is
        # ~0.65us serial per engine); the last group is split across both
        # engines so its data starts moving immediately after the final eq.
        N_OUT_DMA = NG + 1

        # --- input DMAs, dispatched in parallel from both HWDGE engines ---
        nc.sync.dma_start(rowt[:], row_d[:]).then_inc(s_in, 16)
        nc.scalar.dma_start(qts[:], qt_d[:]).then_inc(s_inq, 16)

        # --- PE: broadcast cnd rows via K=1 ones outer products ---
        nc.tensor.wait_ge(s_in, 16)
        for g in range(NG):
            # bc[q, g, :] = ones[q] * cnd_rows[g*512:(g+1)*512]
            nc.tensor.matmul(
                bc[:, g, :],
                lhsT=ones_ap,
                rhs=rowt[0:1, g * GSZ * L : (g + 1) * GSZ * L],
                start=True,
                stop=True,
            ).then_inc(s_pe, 1)

        # --- DVE: qry' prep + wide is_equal per PSUM bank ---
        nc.vector.wait_ge(s_inq, 16)
        # qry' = qry - (qry == 0): zeros become -1. The DVE pipeline has
        # no interlocks: same-engine RAW chains need sem waits.
        nc.vector.tensor_scalar(
            out=eq0[:], in0=qts[:], scalar1=0.0, scalar2=None,
            op0=mybir.AluOpType.is_equal,
        ).then_inc(s_q, 1)
        nc.vector.wait_ge(s_q, 1)
        nc.vector.tensor_tensor(
            out=qp[:], in0=qts[:], in1=eq0[:], op=mybir.AluOpType.subtract,
        ).then_inc(s_q, 1)
        nc.vector.wait_ge(s_q, 2)
        for g in range(NG):
            nc.vector.wait_ge(s_pe, g + 1)
            # out[q, b, c] = (bc[q, b, c] == qry'[q, b])  [b broadcast 128x]
            # AP steps/offsets are in elements; partition dim first.
            in1 = bass.AP(qp, g * GSZ, [[B_LOC, L], [1, GSZ], [0, L]])
            nc.vector.tensor_tensor(
                out=out_sb[:, g * GSZ * L : (g + 1) * GSZ * L].rearrange(
                    "q (b c) -> q b c", b=GSZ
                ),
                in0=bc[:, g, :].rearrange("q (b c) -> q b c", b=GSZ),
                in1=in1,
                op=mybir.AluOpType.is_equal,
            ).then_inc(s_dv, 1)

        # --- output DMAs ---
        nc.sync.wait_ge(s_dv, 1)
        _out_dma(nc.sync, 0, GSZ)
        nc.scalar.wait_ge(s_dv, 2)
        _out_dma(nc.scalar, GSZ, GSZ)
        nc.sync.wait_ge(s_dv, 3)
        _out_dma(nc.sync, 2 * GSZ, GSZ)
        nc.sync.wait_ge(s_dv, 4)
        _out_dma(nc.sync, 3 * GSZ, GSZ // 2)
        nc.scalar.wait_ge(s_dv, 4)
        _out_dma(nc.scalar, 3 * GSZ + GSZ // 2, GSZ // 2)
        nc.sync.wait_ge(s_out, 16 * N_OUT_DMA)

    _strip_barriers(nc, mybir)
    nc.finalize()
    return nc


def _strip_barriers(nc, mybir):
    """Remove bass's const-ap memsets and the entry/exit all-engine
    barriers (~2 us of exec window). All cross-engine ordering in this
    kernel flows through explicit semaphores; the runtime zero-inits
    semaphores at NEFF load, and SP only halts after s_out confirms the
    output DMAs landed, so neither barrier is load-bearing here."""
    f = nc.m.functions[0]
    drop = ("Memset", "Drain", "EventSemaphore")
    for bi, blk in enumerate(f.blocks):
        if blk.name != "main" and not blk.name.endswith("_end"):
            continue
        keep = []
        in_preamble = blk.name == "main"
        for i in blk.instructions:
            if i.opcode == "DMACopy":
                in_preamble = False  # reached kernel body; keep my own waits
            if (in_preamble or blk.name.endswith("_end")) and i.opcode in drop:
                continue
            keep.append(i)
        if len(keep) != len(blk.instructions):
            f.blocks[bi] = mybir.BasicBlock(name=blk.name, instructions=keep)


def _get_nc():
    if "nc" not in _CACHE:
        _CACHE["nc"] = _build_nc()
    return _CACHE["nc"]


def _pack(q, c):
    """Stage per-core inputs: qryT f32 [L, B_LOC] and the cnd+ones row."""
    maps = []
    ones = np.ones((L,), dtype=np.float16)
    for i in range(N_CORES):
        qs = q[i * B_LOC : (i + 1) * B_LOC]
        cs = c[i * B_LOC : (i + 1) * B_LOC]
        qt = np.ascontiguousarray(qs.T.astype(np.float32))
        row = np.concatenate([cs.astype(np.float16).reshape(-1), ones])[None, :]
        maps.append({"qt": qt, "row": np.ascontiguousarray(row)})
    return maps


def _run(q, c, **spmd_kwargs):
    """Shard [B, L] inputs over the 8 cores and run the Bass kernel.

    Returns the BassKernelResults (results per core + optional trace info).
    """
    from concourse.bass_utils import run_bass_kernel_spmd

    nc = _get_nc()
    in_maps = _pack(q, c)
    return run_bass_kernel_spmd(nc, in_maps, core_ids=list(range(N_CORES)), **spmd_kwargs)


def kernel(emb_weight=None, qry_lkup=None, cnd_lkup=None, **_ignored):
    q = np.asarray(qry_lkup, dtype=np.int64)
    c = np.asarray(cnd_lkup, dtype=np.int64)
    assert q.shape == (B, L) and c.shape == (B, L)

    res = _run(q, c)
    out = np.concatenate([r["out"] for r in res.results], axis=0)
    return out
